# revision 4
# baseline (speedup 1.0000x reference)
"""Trainium2 Bass kernel for BLiqNet (liquid-ODE net), 8-core data parallel.

Math (per batch row):
    u  = x @ Wx.T + bx
    dh/dt = (-h + tanh(W h + U u + b)) / tau,  h(0) = u, t in [0, 1]
    y  = h(1) @ Wf.T + bf

Integrator: a single step of ETDRK4 (Cox-Matthews exponential RK4) over
dt = 1.  The linear part L = -1/tau is diagonal, so all phi-function
coefficients are per-hidden-unit vectors, precomputed on the host in fp64.
Measured accuracy vs the 40-step RK4 reference: ~9e-4 relmax (fp16 device
pipeline emulated), far inside the 2e-2 gate.

Device-side restructure ("u-fold"): the latent projection u never
materializes on device.  Writing the stage states s_i, the PSUM-resident
tensor P always equals s_i @ W.T + u @ U.T; stage-to-stage increments are

    P1  = x @ M1.T                      M1 = (W+U) Wx        (K=256)
    t1  = tanh(P + bias1)
    P  += t1 @ Wg.T + x @ M2.T          Wg = W diag(gamma), M2 = W diag(E2-1) Wx
    t2  = tanh(P + bias2)
    P  += (t2 - t1) @ Wg.T
    t3  = tanh(P + bias2)
    P  += ((E2-1) t1 + 2 t3 - t2) @ Wg.T + x @ M3.T
                                        M3 = W diag(E2(E2-1)) Wx
    t4  = tanh(P + bias4)
    hq  = f1 t1 + f2 (t2 + t3) + f3 t4
    yT  = Wf hq + Mh x + cy             Mh = Wf diag(E) Wx   (head, transposed)

with E2 = exp(-1/(2 tau)), E = exp(-1/tau), gamma = (1-E2), f_i the
ETDRK4 phi-combinations over dt=1, and all per-eval constants (b, (W+U)bx,
W diag(.) bx, ...) folded into the tanh bias vectors.  Every matmul is
fp16 x fp16 with a [128,128] stationary tile and N=512 moving columns;
PSUM accumulates fp32.

Layout: hidden dim 512 = 4 tiles x 128 partitions; batch 4096/core as 4
passes x 2 resident 512-column chunks (P = 2 chunks x 4 PSUM banks = all
8 banks; the head reuses chunk bank 0 after the last tanh).  The head is
computed transposed (partitions = 128 outputs, columns = batch) so the
output DMA is layout-direct; the host transposes once at the end.
"""
import numpy as np

import concourse.bass as bass
import concourse.tile as tile
import concourse.mybir as mybir
from concourse import bacc
from concourse import bass_utils

F32 = mybir.dt.float32
F16 = mybir.dt.float16
ALU = mybir.AluOpType
ACTF = mybir.ActivationFunctionType

# problem constants (hardcoded; kernel.py must be self-contained)
B = 32768
IN_DIM = 256
H = 512
OUT_DIM = 128
N_CORES = 8
BL = B // N_CORES          # batch per core = 4096
CHUNK = 512                # batch columns per resident chunk (1 PSUM bank/M-tile)
NCH = 2                    # resident chunks (2*4 = 8 PSUM banks)
BP = CHUNK * NCH           # batch per pass = 1024
PASSES = BL // BP          # 4
HT = H // 128              # 4 hidden tiles
IT = IN_DIM // 128         # 2 input tiles
OT = OUT_DIM // 128        # 1 output tile


def _pack_lhsT(wt):
    """[K, M] lhsT -> [128, (K/128)*(M/128)*128] with tile (kt, mt) at
    columns ((kt*MT)+mt)*128."""
    K, M = wt.shape
    kt, mt = K // 128, M // 128
    return np.ascontiguousarray(
        wt.reshape(kt, 128, mt, 128).transpose(1, 0, 2, 3).reshape(128, kt * mt * 128)
    )


def _pack_pp(v):
    """[H] per-hidden vector -> [128, HT] (column mt holds v[mt*128:(mt+1)*128])."""
    return np.ascontiguousarray(np.asarray(v).reshape(-1, 128).T)


def _build():
    nc = bacc.Bacc("TRN2", target_bir_lowering=False, debug=False,
                   num_devices=N_CORES)

    xT_d = nc.dram_tensor("xT", [IN_DIM, BL], F16, kind="ExternalInput")
    wg_d = nc.dram_tensor("wg", [128, HT * HT * 128], F16, kind="ExternalInput")
    m1_d = nc.dram_tensor("m1", [128, IT * HT * 128], F16, kind="ExternalInput")
    m2_d = nc.dram_tensor("m2", [128, IT * HT * 128], F16, kind="ExternalInput")
    m3_d = nc.dram_tensor("m3", [128, IT * HT * 128], F16, kind="ExternalInput")
    wf_d = nc.dram_tensor("wf", [128, HT * OT * 128], F16, kind="ExternalInput")
    mh_d = nc.dram_tensor("mh", [128, IT * OT * 128], F16, kind="ExternalInput")
    bias1_d = nc.dram_tensor("bias1", [128, HT], F32, kind="ExternalInput")
    bias2_d = nc.dram_tensor("bias2", [128, HT], F32, kind="ExternalInput")
    bias4_d = nc.dram_tensor("bias4", [128, HT], F32, kind="ExternalInput")
    e2m1_d = nc.dram_tensor("e2m1", [128, HT], F32, kind="ExternalInput")
    f1_d = nc.dram_tensor("f1", [128, HT], F32, kind="ExternalInput")
    f2_d = nc.dram_tensor("f2", [128, HT], F32, kind="ExternalInput")
    f3_d = nc.dram_tensor("f3", [128, HT], F32, kind="ExternalInput")
    cy_d = nc.dram_tensor("cy", [128, 1], F32, kind="ExternalInput")
    out_d = nc.dram_tensor("out", [OUT_DIM, BL], F32, kind="ExternalOutput")

    with tile.TileContext(nc) as tc:
        with (
            tc.tile_pool(name="const", bufs=1) as cpool,
            tc.tile_pool(name="work", bufs=1) as wpool,
        ):
            # ---- persistent weights/constants in SBUF ----
            wg_sb = cpool.tile([128, HT * HT * 128], F16)
            m1_sb = cpool.tile([128, IT * HT * 128], F16)
            m2_sb = cpool.tile([128, IT * HT * 128], F16)
            m3_sb = cpool.tile([128, IT * HT * 128], F16)
            wf_sb = cpool.tile([128, HT * OT * 128], F16)
            mh_sb = cpool.tile([128, IT * OT * 128], F16)
            bias1_sb = cpool.tile([128, HT], F32)
            bias2_sb = cpool.tile([128, HT], F32)
            bias4_sb = cpool.tile([128, HT], F32)
            e2m1_sb = cpool.tile([128, HT], F32)
            f1_sb = cpool.tile([128, HT], F32)
            f2_sb = cpool.tile([128, HT], F32)
            f3_sb = cpool.tile([128, HT], F32)
            cy_sb = cpool.tile([128, 1], F32)

            for sb, d in [(m1_sb, m1_d), (wg_sb, wg_d), (bias1_sb, bias1_d),
                          (m2_sb, m2_d), (m3_sb, m3_d), (wf_sb, wf_d),
                          (mh_sb, mh_d), (bias2_sb, bias2_d),
                          (bias4_sb, bias4_d), (e2m1_sb, e2m1_d),
                          (f1_sb, f1_d), (f2_sb, f2_d), (f3_sb, f3_d),
                          (cy_sb, cy_d)]:
                nc.sync.dma_start(sb[:], d.ap())

            def sl(t, mt):
                return t[:, mt * CHUNK:(mt + 1) * CHUNK]

            def mm_h(P_c, w_sb, m_c, start=False):
                """P_c += (W-tile-packed).T @ m_c, K = H (4 kt)."""
                for mt in range(HT):
                    for kt in range(HT):
                        nc.tensor.matmul(
                            sl(P_c, mt),
                            w_sb[:, ((kt * HT) + mt) * 128:((kt * HT) + mt + 1) * 128],
                            sl(m_c, kt),
                            start=(start and kt == 0), stop=(kt == HT - 1),
                            skip_group_check=True,
                        )

            def mm_x(P_c, w_sb, x_c, start=False):
                """P_c += (M-tile-packed).T @ x_c, K = IN_DIM (2 kt)."""
                for mt in range(HT):
                    for kt in range(IT):
                        nc.tensor.matmul(
                            sl(P_c, mt),
                            w_sb[:, ((kt * HT) + mt) * 128:((kt * HT) + mt + 1) * 128],
                            sl(x_c, kt),
                            start=(start and kt == 0), stop=(kt == IT - 1),
                            skip_group_check=True,
                        )

            def tanh_eval(P_c, bias_sb, c, nm):
                t = wpool.tile([128, HT * CHUNK], F16, tag=f"{nm}_{c}",
                               name=f"{nm}_{c}", bufs=1)
                for mt in range(HT):
                    nc.scalar.activation(sl(t, mt), sl(P_c, mt), ACTF.Tanh,
                                         bias=bias_sb[:, mt:mt + 1])
                return t

            for p in range(PASSES):
                with tc.tile_pool(name="ppsum", bufs=1,
                                  space=bass.MemorySpace.PSUM) as ppool:
                    P = [ppool.tile([128, HT * CHUNK], F32, name=f"P{c}")
                         for c in range(NCH)]
                    xs = []
                    t1 = [None] * NCH
                    t2 = [None] * NCH
                    t3 = [None] * NCH

                    # ---- xT slices (fp16, host-transposed) + P1 = x@M1.T ----
                    for c in range(NCH):
                        col0 = p * BP + c * CHUNK
                        xt = wpool.tile([128, IT * CHUNK], F16, tag=f"xt{c}",
                                        name=f"xt{c}", bufs=2)
                        for kt in range(IT):
                            nc.sync.dma_start(
                                xt[:, kt * CHUNK:(kt + 1) * CHUNK],
                                xT_d.ap()[kt * 128:(kt + 1) * 128,
                                          col0:col0 + CHUNK])
                        xs.append(xt)
                    for c in range(NCH):
                        mm_x(P[c], m1_sb, xs[c], start=True)

                    # ---- eval 1 ----
                    for c in range(NCH):
                        t1[c] = tanh_eval(P[c], bias1_sb, c, "t1")
                        mm_h(P[c], wg_sb, t1[c])
                        mm_x(P[c], m2_sb, xs[c])

                    # ---- eval 2 ----
                    for c in range(NCH):
                        t2[c] = tanh_eval(P[c], bias2_sb, c, "t2")
                        m3t = wpool.tile([128, HT * CHUNK], F16, tag=f"m3_{c}",
                                         name=f"m3_{c}", bufs=1)
                        nc.vector.tensor_tensor(m3t[:], t2[c][:], t1[c][:],
                                                op=ALU.subtract)
                        mm_h(P[c], wg_sb, m3t)

                    # ---- eval 3 ----
                    for c in range(NCH):
                        t3[c] = tanh_eval(P[c], bias2_sb, c, "t3")
                        s = wpool.tile([128, HT * CHUNK], F16, tag=f"s_{c}",
                                       name=f"s_{c}", bufs=1)
                        nc.vector.scalar_tensor_tensor(
                            s[:], t3[c][:], 2.0, t2[c][:],
                            op0=ALU.mult, op1=ALU.subtract)
                        v = wpool.tile([128, HT * CHUNK], F16, tag=f"v_{c}",
                                       name=f"v_{c}", bufs=1)
                        for mt in range(HT):
                            nc.vector.scalar_tensor_tensor(
                                sl(v, mt), sl(t1[c], mt),
                                e2m1_sb[:, mt:mt + 1], sl(s, mt),
                                op0=ALU.mult, op1=ALU.add)
                        mm_h(P[c], wg_sb, v)
                        mm_x(P[c], m3_sb, xs[c])

                    # ---- eval 4: hq = f1 t1 + f2 (t2+t3) + f3 t4; head ----
                    for c in range(NCH):
                        t4 = tanh_eval(P[c], bias4_sb, c, "t4")
                        s23 = wpool.tile([128, HT * CHUNK], F16, tag=f"s23_{c}",
                                         name=f"s23_{c}", bufs=1)
                        nc.gpsimd.tensor_tensor(s23[:], t2[c][:], t3[c][:],
                                                op=ALU.add)
                        q = wpool.tile([128, HT * CHUNK], F16, tag=f"q_{c}",
                                       name=f"q_{c}", bufs=1)
                        q2 = wpool.tile([128, HT * CHUNK], F16, tag=f"q2_{c}",
                                        name=f"q2_{c}", bufs=1)
                        hq = wpool.tile([128, HT * CHUNK], F16, tag=f"hq_{c}",
                                        name=f"hq_{c}", bufs=1)
                        for mt in range(HT):
                            nc.scalar.activation(
                                sl(q, mt), sl(t4, mt), ACTF.Identity,
                                scale=f3_sb[:, mt:mt + 1])
                            nc.vector.scalar_tensor_tensor(
                                sl(q2, mt), sl(s23, mt),
                                f2_sb[:, mt:mt + 1], sl(q, mt),
                                op0=ALU.mult, op1=ALU.add)
                            nc.vector.scalar_tensor_tensor(
                                sl(hq, mt), sl(t1[c], mt),
                                f1_sb[:, mt:mt + 1], sl(q2, mt),
                                op0=ALU.mult, op1=ALU.add)

                        # head into P[c] bank 0 (dead after t4 read)
                        yT = P[c][:OUT_DIM, 0:CHUNK]
                        for kt in range(HT):
                            nc.tensor.matmul(
                                yT, wf_sb[:, kt * 128:(kt + 1) * 128],
                                sl(hq, kt), start=(kt == 0), stop=False,
                                skip_group_check=True)
                        for kt in range(IT):
                            nc.tensor.matmul(
                                yT, mh_sb[:, kt * 128:(kt + 1) * 128],
                                xs[c][:, kt * CHUNK:(kt + 1) * CHUNK],
                                start=False, stop=(kt == IT - 1),
                                skip_group_check=True)
                        ob = wpool.tile([OUT_DIM, CHUNK], F32, tag=f"ob{c}",
                                        name=f"ob{c}", bufs=2)
                        nc.scalar.activation(ob[:], yT, ACTF.Identity,
                                             bias=cy_sb[:, 0:1])
                        col0 = p * BP + c * CHUNK
                        nc.sync.dma_start(out_d.ap()[:, col0:col0 + CHUNK],
                                          ob[:])

    nc.compile()
    return nc


_CACHED = None
RUN_KWARGS = {}
LAST_RESULT = None


def _get_nc():
    global _CACHED
    if _CACHED is None:
        _CACHED = _build()
    return _CACHED


def kernel(x, Wx, bx, W, U, b, tau, Wf, bf):
    x = np.asarray(x, np.float32)
    Wx = np.asarray(Wx, np.float64)
    bx = np.asarray(bx, np.float64)
    W = np.asarray(W, np.float64)
    U = np.asarray(U, np.float64)
    b = np.asarray(b, np.float64)
    tau = np.asarray(tau, np.float64)
    Wf = np.asarray(Wf, np.float64)
    bf = np.asarray(bf, np.float64)

    # ---- ETDRK4 (Cox-Matthews) coefficients, dt = 1, L = -1/tau ----
    z = -1.0 / tau
    z2 = 0.5 * z
    E2 = np.exp(z2)
    E = np.exp(z)

    def phi1(v): return np.expm1(v) / v
    def phi2(v): return (np.expm1(v) - v) / v**2
    def phi3(v): return (np.expm1(v) - v - v * v / 2) / v**3

    gam = 0.5 * phi1(z2) / tau          # == 1 - E2
    f1 = (phi1(z) - 3 * phi2(z) + 4 * phi3(z)) / tau
    f2 = (2 * phi2(z) - 4 * phi3(z)) / tau
    f3 = (-phi2(z) + 4 * phi3(z)) / tau
    e2m1 = E2 - 1.0

    M1 = (W + U) @ Wx
    c1 = (W + U) @ bx
    M2 = (W * e2m1[None, :]) @ Wx
    c2 = W @ (e2m1 * bx)
    M3 = (W * (E2 * e2m1)[None, :]) @ Wx
    c3 = W @ (E2 * e2m1 * bx)
    Mh = (Wf * E[None, :]) @ Wx
    cy = Wf @ (E * bx) + bf
    Wg = W * gam[None, :]

    bias1 = b + c1
    bias2 = bias1 + c2
    bias4 = bias2 + c3

    weights = {
        "wg": _pack_lhsT(Wg.T.astype(np.float16)),
        "m1": _pack_lhsT(M1.T.astype(np.float16)),
        "m2": _pack_lhsT(M2.T.astype(np.float16)),
        "m3": _pack_lhsT(M3.T.astype(np.float16)),
        "wf": _pack_lhsT(Wf.T.astype(np.float16)),
        "mh": _pack_lhsT(Mh.T.astype(np.float16)),
        "bias1": _pack_pp(bias1.astype(np.float32)),
        "bias2": _pack_pp(bias2.astype(np.float32)),
        "bias4": _pack_pp(bias4.astype(np.float32)),
        "e2m1": _pack_pp(e2m1.astype(np.float32)),
        "f1": _pack_pp(f1.astype(np.float32)),
        "f2": _pack_pp(f2.astype(np.float32)),
        "f3": _pack_pp(f3.astype(np.float32)),
        "cy": np.ascontiguousarray(cy.astype(np.float32).reshape(128, 1)),
    }

    nc = _get_nc()
    in_maps = []
    for c in range(N_CORES):
        m = dict(weights)
        m["xT"] = np.ascontiguousarray(
            x[c * BL:(c + 1) * BL].T.astype(np.float16))
        in_maps.append(m)
    res = bass_utils.run_bass_kernel_spmd(nc, in_maps,
                                          core_ids=list(range(N_CORES)),
                                          **RUN_KWARGS)
    global LAST_RESULT
    LAST_RESULT = res
    return np.ascontiguousarray(
        np.concatenate([res.results[c]["out"].T for c in range(N_CORES)],
                       axis=0))


# revision 5
# speedup vs baseline: 1.5644x; 1.5644x over previous
"""Trainium2 Bass kernel for BLiqNet (liquid-ODE net), 8-core data parallel.

Math (per batch row):
    u  = x @ Wx.T + bx
    dh/dt = (-h + tanh(W h + U u + b)) / tau,  h(0) = u, t in [0, 1]
    y  = h(1) @ Wf.T + bf

Integrator: a single step of ETDRK3 (Cox-Matthews exponential RK3) over
dt = 1.  The linear part L = -1/tau is diagonal, so all phi-function
coefficients are per-hidden-unit vectors, precomputed on the host in
fp64.  Accuracy vs the 40-step RK4 reference: ~4.5e-3 relmax (fp16
device pipeline emulated; measured ETDRK4 variant matched its emulation
within 1e-4), inside the 2e-2 gate with 4x margin.

Device-side restructure ("u-fold"): the latent projection u never
materializes on device.  With stage states s_i, the PSUM-resident tensor
P always equals s_i @ W.T + u @ U.T:

    P1  = x @ M1.T                       M1 = (W+U) Wx          (K=256)
    t1  = tanh(P + bias1)
    P  += t1 @ Wg2.T + x @ M2.T          Wg2 = W diag(gam2)
                                         M2  = W diag(E2-1) Wx
    t2  = tanh(P + bias2)
    P  += (2 t2 - r t1) @ Wg1.T + x @ M3.T
                                         Wg1 = W diag(gam1)
                                         M3  = W diag(E-E2) Wx, r = (gam1+gam2)/gam1
    t3  = tanh(P + bias3)
    d   = g2 t2 + g3 t3                  (broadcast-constant tensor_tensor)
    yT  = Wf d + Wf1 t1 + Mh x + cy      Wf1 = Wf diag(g1), Mh = Wf diag(E) Wx

with E2 = exp(-1/(2 tau)), E = exp(-1/tau), gam2 = phi1(z/2)/(2 tau),
gam1 = phi1(z)/tau, g1..g3 the ETDRK3 output weights over dt=1, and all
per-eval constants folded into the tanh bias vectors.  Every matmul is
fp16 x fp16 with a [128,128] stationary tile and N=512 moving columns;
PSUM accumulates fp32.  Elementwise work is plain tensor_tensor on the
DVE (scalar_tensor_tensor and gpsimd measured 3-5x slower); per-unit
constants ship as host-precomputed broadcast tiles.

Layout: hidden 512 = 4 tiles x 128 partitions; batch 4096/core as 8
chunks of 512 columns, two in flight (P = 2 x 4 PSUM banks = all 8
banks, allocated from one long-lived pool with rotating tags so pass
boundaries only serialize per-chunk).  The head reuses chunk bank 0
after the last tanh read.  The head is computed transposed (partitions =
128 outputs, columns = batch) so the output DMA is layout-direct; the
host transposes once at the end.
"""
import numpy as np

import concourse.bass as bass
import concourse.tile as tile
import concourse.mybir as mybir
from concourse import bacc
from concourse import bass_utils

F32 = mybir.dt.float32
F16 = mybir.dt.float16
ALU = mybir.AluOpType
ACTF = mybir.ActivationFunctionType

# problem constants (hardcoded; kernel.py must be self-contained)
B = 32768
IN_DIM = 256
H = 512
OUT_DIM = 128
N_CORES = 8
BL = B // N_CORES          # batch per core = 4096
CHUNK = 512                # batch columns per resident chunk (1 PSUM bank/M-tile)
NCH = 2                    # resident chunks (2*4 = 8 PSUM banks)
BP = CHUNK * NCH           # batch per pass = 1024
PASSES = BL // BP          # 4
HT = H // 128              # 4 hidden tiles
IT = IN_DIM // 128         # 2 input tiles


def _pack_lhsT(wt):
    """[K, M] lhsT -> [128, (K/128)*(M/128)*128] with tile (kt, mt) at
    columns ((kt*MT)+mt)*128."""
    K, M = wt.shape
    kt, mt = K // 128, M // 128
    return np.ascontiguousarray(
        wt.reshape(kt, 128, mt, 128).transpose(1, 0, 2, 3).reshape(128, kt * mt * 128)
    )


def _pack_pp(v):
    """[H] per-hidden vector -> [128, HT] (column mt holds v[mt*128:(mt+1)*128])."""
    return np.ascontiguousarray(np.asarray(v).reshape(-1, 128).T)


def _bcast(v):
    """[H] per-hidden vector -> [128, HT*CHUNK] fp16 broadcast tile."""
    pp = _pack_pp(v)                       # [128, HT]
    return np.ascontiguousarray(
        np.repeat(pp, CHUNK, axis=1).astype(np.float16))


def _build():
    nc = bacc.Bacc("TRN2", target_bir_lowering=False, debug=False,
                   num_devices=N_CORES)

    xT_d = nc.dram_tensor("xT", [IN_DIM, BL], F16, kind="ExternalInput")
    wg1_d = nc.dram_tensor("wg1", [128, HT * HT * 128], F16, kind="ExternalInput")
    wg2_d = nc.dram_tensor("wg2", [128, HT * HT * 128], F16, kind="ExternalInput")
    m1_d = nc.dram_tensor("m1", [128, IT * HT * 128], F16, kind="ExternalInput")
    m2_d = nc.dram_tensor("m2", [128, IT * HT * 128], F16, kind="ExternalInput")
    m3_d = nc.dram_tensor("m3", [128, IT * HT * 128], F16, kind="ExternalInput")
    wf_d = nc.dram_tensor("wf", [128, HT * 128], F16, kind="ExternalInput")
    wf1_d = nc.dram_tensor("wf1", [128, HT * 128], F16, kind="ExternalInput")
    mh_d = nc.dram_tensor("mh", [128, IT * 128], F16, kind="ExternalInput")
    bias1_d = nc.dram_tensor("bias1", [128, HT], F32, kind="ExternalInput")
    bias2_d = nc.dram_tensor("bias2", [128, HT], F32, kind="ExternalInput")
    bias3_d = nc.dram_tensor("bias3", [128, HT], F32, kind="ExternalInput")
    rbc_d = nc.dram_tensor("rbc", [128, HT * CHUNK], F16, kind="ExternalInput")
    g2bc_d = nc.dram_tensor("g2bc", [128, HT * CHUNK], F16, kind="ExternalInput")
    g3bc_d = nc.dram_tensor("g3bc", [128, HT * CHUNK], F16, kind="ExternalInput")
    cy_d = nc.dram_tensor("cy", [128, 1], F32, kind="ExternalInput")
    out_d = nc.dram_tensor("out", [OUT_DIM, BL], F32, kind="ExternalOutput")

    with tile.TileContext(nc) as tc:
        with (
            tc.tile_pool(name="const", bufs=1) as cpool,
            tc.tile_pool(name="work", bufs=1) as wpool,
            tc.tile_pool(name="ppsum", bufs=1,
                         space=bass.MemorySpace.PSUM) as ppool,
        ):
            # ---- persistent weights/constants in SBUF ----
            wg1_sb = cpool.tile([128, HT * HT * 128], F16)
            wg2_sb = cpool.tile([128, HT * HT * 128], F16)
            m1_sb = cpool.tile([128, IT * HT * 128], F16)
            m2_sb = cpool.tile([128, IT * HT * 128], F16)
            m3_sb = cpool.tile([128, IT * HT * 128], F16)
            wf_sb = cpool.tile([128, HT * 128], F16)
            wf1_sb = cpool.tile([128, HT * 128], F16)
            mh_sb = cpool.tile([128, IT * 128], F16)
            bias1_sb = cpool.tile([128, HT], F32)
            bias2_sb = cpool.tile([128, HT], F32)
            bias3_sb = cpool.tile([128, HT], F32)
            rbc_sb = cpool.tile([128, HT * CHUNK], F16)
            g2bc_sb = cpool.tile([128, HT * CHUNK], F16)
            g3bc_sb = cpool.tile([128, HT * CHUNK], F16)
            cy_sb = cpool.tile([128, 1], F32)

            for sb, d in [(m1_sb, m1_d), (bias1_sb, bias1_d),
                          (wg2_sb, wg2_d), (m2_sb, m2_d), (bias2_sb, bias2_d),
                          (rbc_sb, rbc_d), (wg1_sb, wg1_d), (m3_sb, m3_d),
                          (bias3_sb, bias3_d), (g2bc_sb, g2bc_d),
                          (g3bc_sb, g3bc_d), (wf_sb, wf_d), (wf1_sb, wf1_d),
                          (mh_sb, mh_d), (cy_sb, cy_d)]:
                nc.sync.dma_start(sb[:], d.ap())

            def sl(t, mt):
                return t[:, mt * CHUNK:(mt + 1) * CHUNK]

            def mm_h(P_c, w_sb, m_c, start=False):
                """P_c += (W-tile-packed).T @ m_c, K = H (4 kt)."""
                for mt in range(HT):
                    for kt in range(HT):
                        nc.tensor.matmul(
                            sl(P_c, mt),
                            w_sb[:, ((kt * HT) + mt) * 128:((kt * HT) + mt + 1) * 128],
                            sl(m_c, kt),
                            start=(start and kt == 0), stop=(kt == HT - 1),
                            skip_group_check=True,
                        )

            def mm_x(P_c, w_sb, x_c, start=False):
                """P_c += (M-tile-packed).T @ x_c, K = IN_DIM (2 kt)."""
                for mt in range(HT):
                    for kt in range(IT):
                        nc.tensor.matmul(
                            sl(P_c, mt),
                            w_sb[:, ((kt * HT) + mt) * 128:((kt * HT) + mt + 1) * 128],
                            sl(x_c, kt),
                            start=(start and kt == 0), stop=(kt == IT - 1),
                            skip_group_check=True,
                        )

            def tanh_eval(P_c, bias_sb, c, nm):
                t = wpool.tile([128, HT * CHUNK], F16, tag=f"{nm}_{c}",
                               name=f"{nm}_{c}", bufs=1)
                for mt in range(HT):
                    nc.scalar.activation(sl(t, mt), sl(P_c, mt), ACTF.Tanh,
                                         bias=bias_sb[:, mt:mt + 1])
                return t

            for p in range(PASSES):
                P = [ppool.tile([128, HT * CHUNK], F32, tag=f"P{c}",
                                name=f"P{c}", bufs=1) for c in range(NCH)]
                xs = []
                t1 = [None] * NCH
                t2 = [None] * NCH

                for c in range(NCH):
                    col0 = p * BP + c * CHUNK
                    xt = wpool.tile([128, IT * CHUNK], F16, tag=f"xt{c}",
                                    name=f"xt{c}", bufs=2)
                    for kt in range(IT):
                        nc.sync.dma_start(
                            xt[:, kt * CHUNK:(kt + 1) * CHUNK],
                            xT_d.ap()[kt * 128:(kt + 1) * 128,
                                      col0:col0 + CHUNK])
                    xs.append(xt)
                for c in range(NCH):
                    mm_x(P[c], m1_sb, xs[c], start=True)

                # ---- eval 1 ----
                for c in range(NCH):
                    t1[c] = tanh_eval(P[c], bias1_sb, c, "t1")
                    mm_h(P[c], wg2_sb, t1[c])
                    mm_x(P[c], m2_sb, xs[c])

                # ---- eval 2: v2 = 2 t2 - r t1 ----
                for c in range(NCH):
                    t2[c] = tanh_eval(P[c], bias2_sb, c, "t2")
                    a = wpool.tile([128, HT * CHUNK], F16, tag=f"a_{c}",
                                   name=f"a_{c}", bufs=1)
                    v = wpool.tile([128, HT * CHUNK], F16, tag=f"v_{c}",
                                   name=f"v_{c}", bufs=1)
                    v2 = wpool.tile([128, HT * CHUNK], F16, tag=f"v2_{c}",
                                    name=f"v2_{c}", bufs=1)
                    nc.vector.tensor_tensor(a[:], t1[c][:], rbc_sb[:],
                                            op=ALU.mult)
                    nc.vector.tensor_tensor(v[:], t2[c][:], t2[c][:],
                                            op=ALU.add)
                    nc.vector.tensor_tensor(v2[:], v[:], a[:],
                                            op=ALU.subtract)
                    mm_h(P[c], wg1_sb, v2)
                    mm_x(P[c], m3_sb, xs[c])

                # ---- eval 3 + head ----
                for c in range(NCH):
                    t3 = tanh_eval(P[c], bias3_sb, c, "t3")
                    a2 = wpool.tile([128, HT * CHUNK], F16, tag=f"a2_{c}",
                                    name=f"a2_{c}", bufs=1)
                    a3 = wpool.tile([128, HT * CHUNK], F16, tag=f"a3_{c}",
                                    name=f"a3_{c}", bufs=1)
                    d = wpool.tile([128, HT * CHUNK], F16, tag=f"d_{c}",
                                   name=f"d_{c}", bufs=1)
                    nc.vector.tensor_tensor(a2[:], t2[c][:], g2bc_sb[:],
                                            op=ALU.mult)
                    nc.vector.tensor_tensor(a3[:], t3[:], g3bc_sb[:],
                                            op=ALU.mult)
                    nc.vector.tensor_tensor(d[:], a2[:], a3[:], op=ALU.add)

                    # head into P[c] bank 0 (dead after t3 read)
                    yT = P[c][:OUT_DIM, 0:CHUNK]
                    for kt in range(HT):
                        nc.tensor.matmul(
                            yT, wf_sb[:, kt * 128:(kt + 1) * 128],
                            sl(d, kt), start=(kt == 0), stop=False,
                            skip_group_check=True)
                    for kt in range(HT):
                        nc.tensor.matmul(
                            yT, wf1_sb[:, kt * 128:(kt + 1) * 128],
                            sl(t1[c], kt), start=False, stop=False,
                            skip_group_check=True)
                    for kt in range(IT):
                        nc.tensor.matmul(
                            yT, mh_sb[:, kt * 128:(kt + 1) * 128],
                            xs[c][:, kt * CHUNK:(kt + 1) * CHUNK],
                            start=False, stop=(kt == IT - 1),
                            skip_group_check=True)
                    ob = wpool.tile([OUT_DIM, CHUNK], F32, tag=f"ob{c}",
                                    name=f"ob{c}", bufs=2)
                    nc.scalar.activation(ob[:], yT, ACTF.Identity,
                                         bias=cy_sb[:, 0:1])
                    col0 = p * BP + c * CHUNK
                    nc.sync.dma_start(out_d.ap()[:, col0:col0 + CHUNK],
                                      ob[:])

    nc.compile()
    return nc


_CACHED = None
RUN_KWARGS = {}
LAST_RESULT = None


def _get_nc():
    global _CACHED
    if _CACHED is None:
        _CACHED = _build()
    return _CACHED


def kernel(x, Wx, bx, W, U, b, tau, Wf, bf):
    x = np.asarray(x, np.float32)
    Wx = np.asarray(Wx, np.float64)
    bx = np.asarray(bx, np.float64)
    W = np.asarray(W, np.float64)
    U = np.asarray(U, np.float64)
    b = np.asarray(b, np.float64)
    tau = np.asarray(tau, np.float64)
    Wf = np.asarray(Wf, np.float64)
    bf = np.asarray(bf, np.float64)

    # ---- ETDRK3 (Cox-Matthews) coefficients, dt = 1, L = -1/tau ----
    z = -1.0 / tau
    z2 = 0.5 * z
    E2 = np.exp(z2)
    E = np.exp(z)

    def phi1(v): return np.expm1(v) / v
    def phi2(v): return (np.expm1(v) - v) / v**2
    def phi3(v): return (np.expm1(v) - v - v * v / 2) / v**3

    gam2 = 0.5 * phi1(z2) / tau
    gam1 = phi1(z) / tau
    g1 = (4 * phi3(z) - 3 * phi2(z) + phi1(z)) / tau
    g2 = (4 * phi2(z) - 8 * phi3(z)) / tau
    g3 = (4 * phi3(z) - phi2(z)) / tau

    M1 = (W + U) @ Wx
    c1 = (W + U) @ bx
    M2 = (W * (E2 - 1.0)[None, :]) @ Wx
    c2 = W @ ((E2 - 1.0) * bx)
    M3 = (W * (E - E2)[None, :]) @ Wx
    c3 = W @ ((E - E2) * bx)
    Mh = (Wf * E[None, :]) @ Wx
    cy = Wf @ (E * bx) + bf
    Wg2 = W * gam2[None, :]
    Wg1 = W * gam1[None, :]
    Wf1 = Wf * g1[None, :]
    r = (gam1 + gam2) / gam1

    bias1 = b + c1
    bias2 = bias1 + c2
    bias3 = bias2 + c3

    weights = {
        "wg1": _pack_lhsT(Wg1.T.astype(np.float16)),
        "wg2": _pack_lhsT(Wg2.T.astype(np.float16)),
        "m1": _pack_lhsT(M1.T.astype(np.float16)),
        "m2": _pack_lhsT(M2.T.astype(np.float16)),
        "m3": _pack_lhsT(M3.T.astype(np.float16)),
        "wf": _pack_lhsT(Wf.T.astype(np.float16)),
        "wf1": _pack_lhsT(Wf1.T.astype(np.float16)),
        "mh": _pack_lhsT(Mh.T.astype(np.float16)),
        "bias1": _pack_pp(bias1.astype(np.float32)),
        "bias2": _pack_pp(bias2.astype(np.float32)),
        "bias3": _pack_pp(bias3.astype(np.float32)),
        "rbc": _bcast(r),
        "g2bc": _bcast(g2),
        "g3bc": _bcast(g3),
        "cy": np.ascontiguousarray(cy.astype(np.float32).reshape(128, 1)),
    }

    nc = _get_nc()
    in_maps = []
    for c in range(N_CORES):
        m = dict(weights)
        m["xT"] = np.ascontiguousarray(
            x[c * BL:(c + 1) * BL].T.astype(np.float16))
        in_maps.append(m)
    res = bass_utils.run_bass_kernel_spmd(nc, in_maps,
                                          core_ids=list(range(N_CORES)),
                                          **RUN_KWARGS)
    global LAST_RESULT
    LAST_RESULT = res
    return np.ascontiguousarray(
        np.concatenate([res.results[c]["out"].T for c in range(N_CORES)],
                       axis=0))


# revision 16
# speedup vs baseline: 1.6771x; 1.0720x over previous
"""Trainium2 Bass kernel for BLiqNet (liquid-ODE net), 8-core data parallel.

Math (per batch row):
    u  = x @ Wx.T + bx
    dh/dt = (-h + tanh(W h + U u + b)) / tau,  h(0) = u, t in [0, 1]
    y  = h(1) @ Wf.T + bf

Integrator: a single step of ETDRK3 (Cox-Matthews exponential RK3) over
dt = 1.  The linear part L = -1/tau is diagonal, so all phi-function
coefficients are per-hidden-unit vectors, precomputed on the host in
fp64.  Accuracy vs the 40-step RK4 reference: ~4.5e-3 relmax (fp16
device pipeline emulated; measured ETDRK4 variant matched its emulation
within 1e-4), inside the 2e-2 gate with 4x margin.

Device-side restructure ("u-fold"): the latent projection u never
materializes on device.  With stage states s_i, the PSUM-resident tensor
P always equals s_i @ W.T + u @ U.T:

    P1  = x @ M1.T                       M1 = (W+U) Wx          (K=256)
    t1  = tanh(P + bias1)
    P  += t1 @ Wg2.T + x @ M2.T          Wg2 = W diag(gam2)
                                         M2  = W diag(E2-1) Wx
    t2  = tanh(P + bias2)
    P  += (2 t2 - r t1) @ Wg1.T + x @ M3.T
                                         Wg1 = W diag(gam1)
                                         M3  = W diag(E-E2) Wx, r = (gam1+gam2)/gam1
    t3  = tanh(P + bias3)
    d   = g2 t2 + g3 t3                  (broadcast-constant tensor_tensor)
    yT  = Wf d + Wf1 t1 + Mh x + cy      Wf1 = Wf diag(g1), Mh = Wf diag(E) Wx

with E2 = exp(-1/(2 tau)), E = exp(-1/tau), gam2 = phi1(z/2)/(2 tau),
gam1 = phi1(z)/tau, g1..g3 the ETDRK3 output weights over dt=1, and all
per-eval constants folded into the tanh bias vectors.  Every matmul is
fp16 x fp16 with a [128,128] stationary tile and N=512 moving columns;
PSUM accumulates fp32.  Elementwise work is plain tensor_tensor on the
DVE (scalar_tensor_tensor and gpsimd measured 3-5x slower); per-unit
constants ship as host-precomputed broadcast tiles.

Layout: hidden 512 = 4 tiles x 128 partitions; batch 4096/core as 8
chunks of 512 columns, two in flight (P = 2 x 4 PSUM banks = all 8
banks, allocated from one long-lived pool with rotating tags so pass
boundaries only serialize per-chunk).  The head reuses chunk bank 0
after the last tanh read.  The head is computed transposed (partitions =
128 outputs, columns = batch) so the output DMA is layout-direct; the
host transposes once at the end.
"""
import numpy as np

import concourse.bass as bass
import concourse.tile as tile
import concourse.mybir as mybir
from concourse import bacc
from concourse import bass_utils

F32 = mybir.dt.float32
F16 = mybir.dt.float16
ALU = mybir.AluOpType
ACTF = mybir.ActivationFunctionType

# problem constants (hardcoded; kernel.py must be self-contained)
B = 32768
IN_DIM = 256
H = 512
OUT_DIM = 128
N_CORES = 8
BL = B // N_CORES          # batch per core = 4096
CHUNK = 512                # batch columns per resident chunk (1 PSUM bank/M-tile)
NCH = 2                    # resident chunks (2*4 = 8 PSUM banks)
BP = CHUNK * NCH           # batch per pass = 1024
PASSES = BL // BP          # 4
HT = H // 128              # 4 hidden tiles
IT = IN_DIM // 128         # 2 input tiles


def _pack_lhsT(wt):
    """[K, M] lhsT -> [128, (K/128)*(M/128)*128] with tile (kt, mt) at
    columns ((kt*MT)+mt)*128."""
    K, M = wt.shape
    kt, mt = K // 128, M // 128
    return np.ascontiguousarray(
        wt.reshape(kt, 128, mt, 128).transpose(1, 0, 2, 3).reshape(128, kt * mt * 128)
    )


def _pack_pp(v):
    """[H] per-hidden vector -> [128, HT] (column mt holds v[mt*128:(mt+1)*128])."""
    return np.ascontiguousarray(np.asarray(v).reshape(-1, 128).T)


def _bcast(v):
    """[H] per-hidden vector -> [128, HT*CHUNK] fp16 broadcast tile."""
    pp = _pack_pp(v)                       # [128, HT]
    return np.ascontiguousarray(
        np.repeat(pp, CHUNK, axis=1).astype(np.float16))


def _build():
    nc = bacc.Bacc("TRN2", target_bir_lowering=False, debug=False,
                   num_devices=N_CORES)

    xT_d = nc.dram_tensor("xT", [IN_DIM, BL], F16, kind="ExternalInput")
    wg1_d = nc.dram_tensor("wg1", [128, HT * HT * 128], F16, kind="ExternalInput")
    wg2_d = nc.dram_tensor("wg2", [128, HT * HT * 128], F16, kind="ExternalInput")
    m1_d = nc.dram_tensor("m1", [128, IT * HT * 128], F16, kind="ExternalInput")
    m2_d = nc.dram_tensor("m2", [128, IT * HT * 128], F16, kind="ExternalInput")
    m3_d = nc.dram_tensor("m3", [128, IT * HT * 128], F16, kind="ExternalInput")
    wf_d = nc.dram_tensor("wf", [128, HT * 128], F16, kind="ExternalInput")
    wf1_d = nc.dram_tensor("wf1", [128, HT * 128], F16, kind="ExternalInput")
    mh_d = nc.dram_tensor("mh", [128, IT * 128], F16, kind="ExternalInput")
    bias1_d = nc.dram_tensor("bias1", [128, HT], F32, kind="ExternalInput")
    bias2_d = nc.dram_tensor("bias2", [128, HT], F32, kind="ExternalInput")
    bias3_d = nc.dram_tensor("bias3", [128, HT], F32, kind="ExternalInput")
    rbc_d = nc.dram_tensor("rbc", [128, HT * CHUNK], F16, kind="ExternalInput")
    g3bc_d = nc.dram_tensor("g3bc", [128, HT * CHUNK], F16, kind="ExternalInput")
    cy_d = nc.dram_tensor("cy", [128, 1], F32, kind="ExternalInput")
    out_d = nc.dram_tensor("out", [OUT_DIM, BL], F32, kind="ExternalOutput")

    with tile.TileContext(nc) as tc:
        with (
            tc.tile_pool(name="const", bufs=1) as cpool,
            tc.tile_pool(name="work", bufs=1) as wpool,
            tc.tile_pool(name="ppsum", bufs=1,
                         space=bass.MemorySpace.PSUM) as ppool,
        ):
            # ---- persistent weights/constants in SBUF ----
            wg1_sb = cpool.tile([128, HT * HT * 128], F16)
            wg2_sb = cpool.tile([128, HT * HT * 128], F16)
            m1_sb = cpool.tile([128, IT * HT * 128], F16)
            m2_sb = cpool.tile([128, IT * HT * 128], F16)
            m3_sb = cpool.tile([128, IT * HT * 128], F16)
            wf_sb = cpool.tile([128, HT * 128], F16)
            wf1_sb = cpool.tile([128, HT * 128], F16)
            mh_sb = cpool.tile([128, IT * 128], F16)
            bias1_sb = cpool.tile([128, HT], F32)
            bias2_sb = cpool.tile([128, HT], F32)
            bias3_sb = cpool.tile([128, HT], F32)
            rbc_sb = cpool.tile([128, HT * CHUNK], F16)
            g3bc_sb = cpool.tile([128, HT * CHUNK], F16)
            cy_sb = cpool.tile([128, 1], F32)

            for sb, d in [(m1_sb, m1_d), (bias1_sb, bias1_d),
                          (wg2_sb, wg2_d), (m2_sb, m2_d), (bias2_sb, bias2_d),
                          (wg1_sb, wg1_d), (rbc_sb, rbc_d), (m3_sb, m3_d),
                          (bias3_sb, bias3_d), (g3bc_sb, g3bc_d),
                          (wf_sb, wf_d), (wf1_sb, wf1_d),
                          (mh_sb, mh_d), (cy_sb, cy_d)]:
                nc.sync.dma_start(sb[:], d.ap())

            def sl(t, mt):
                return t[:, mt * CHUNK:(mt + 1) * CHUNK]

            def mm_h(P_c, w_sb, m_c, start=False):
                """P_c += (W-tile-packed).T @ m_c, K = H (4 kt)."""
                for mt in range(HT):
                    for kt in range(HT):
                        nc.tensor.matmul(
                            sl(P_c, mt),
                            w_sb[:, ((kt * HT) + mt) * 128:((kt * HT) + mt + 1) * 128],
                            sl(m_c, kt),
                            start=(start and kt == 0), stop=(kt == HT - 1),
                            skip_group_check=True,
                        )

            def mm_x(P_c, w_sb, x_c, start=False):
                """P_c += (M-tile-packed).T @ x_c, K = IN_DIM (2 kt)."""
                for mt in range(HT):
                    for kt in range(IT):
                        nc.tensor.matmul(
                            sl(P_c, mt),
                            w_sb[:, ((kt * HT) + mt) * 128:((kt * HT) + mt + 1) * 128],
                            sl(x_c, kt),
                            start=(start and kt == 0), stop=(kt == IT - 1),
                            skip_group_check=True,
                        )

            def tanh_eval(P_c, bias_sb, c, nm):
                t = wpool.tile([128, HT * CHUNK], F16, tag=f"{nm}_{c}",
                               name=f"{nm}_{c}", bufs=1)
                for mt in range(HT):
                    nc.scalar.activation(sl(t, mt), sl(P_c, mt), ACTF.Tanh,
                                         bias=bias_sb[:, mt:mt + 1])
                return t

            for p in range(PASSES):
                P = [ppool.tile([128, HT * CHUNK], F32, tag=f"P{c}",
                                name=f"P{c}", bufs=1) for c in range(NCH)]
                xs = []
                t1 = [None] * NCH
                t2 = [None] * NCH

                for c in range(NCH):
                    col0 = p * BP + c * CHUNK
                    xt = wpool.tile([128, IT * CHUNK], F16, tag=f"xt{c}",
                                    name=f"xt{c}", bufs=2)
                    for kt in range(IT):
                        # gpsimd queue: x loads issue in parallel with the
                        # const DMAs on the sync queue
                        nc.gpsimd.dma_start(
                            xt[:, kt * CHUNK:(kt + 1) * CHUNK],
                            xT_d.ap()[kt * 128:(kt + 1) * 128,
                                      col0:col0 + CHUNK])
                    xs.append(xt)
                for c in range(NCH):
                    mm_x(P[c], m1_sb, xs[c], start=True)

                # ---- eval 1 ----
                for c in range(NCH):
                    t1[c] = tanh_eval(P[c], bias1_sb, c, "t1")
                    mm_h(P[c], wg2_sb, t1[c])
                    mm_x(P[c], m2_sb, xs[c])

                # ---- eval 2: v2 = t2 - (r/2) t1  (wg1 holds 2*Wg1) ----
                for c in range(NCH):
                    t2[c] = tanh_eval(P[c], bias2_sb, c, "t2")
                    a = wpool.tile([128, HT * CHUNK], F16, tag=f"a_{c}",
                                   name=f"a_{c}", bufs=1)
                    v2 = wpool.tile([128, HT * CHUNK], F16, tag=f"v2_{c}",
                                    name=f"v2_{c}", bufs=1)
                    nc.vector.tensor_tensor(a[:], t1[c][:], rbc_sb[:],
                                            op=ALU.mult)
                    nc.vector.tensor_tensor(v2[:], t2[c][:], a[:],
                                            op=ALU.subtract)
                    mm_h(P[c], wg1_sb, v2)
                    mm_x(P[c], m3_sb, xs[c])

                # ---- eval 3 + head ----
                for c in range(NCH):
                    t3 = tanh_eval(P[c], bias3_sb, c, "t3")
                    a3 = wpool.tile([128, HT * CHUNK], F16, tag=f"a3_{c}",
                                    name=f"a3_{c}", bufs=1)
                    d = wpool.tile([128, HT * CHUNK], F16, tag=f"d_{c}",
                                   name=f"d_{c}", bufs=1)
                    # d = t2 + (g3/g2) t3  (wf holds Wf diag(g2))
                    nc.vector.tensor_tensor(a3[:], t3[:], g3bc_sb[:],
                                            op=ALU.mult)
                    nc.vector.tensor_tensor(d[:], t2[c][:], a3[:], op=ALU.add)

                    # head into P[c] bank 0 (dead after t3 read)
                    yT = P[c][:OUT_DIM, 0:CHUNK]
                    for kt in range(HT):
                        nc.tensor.matmul(
                            yT, wf_sb[:, kt * 128:(kt + 1) * 128],
                            sl(d, kt), start=(kt == 0), stop=False,
                            skip_group_check=True)
                    for kt in range(HT):
                        nc.tensor.matmul(
                            yT, wf1_sb[:, kt * 128:(kt + 1) * 128],
                            sl(t1[c], kt), start=False, stop=False,
                            skip_group_check=True)
                    for kt in range(IT):
                        nc.tensor.matmul(
                            yT, mh_sb[:, kt * 128:(kt + 1) * 128],
                            xs[c][:, kt * CHUNK:(kt + 1) * CHUNK],
                            start=False, stop=(kt == IT - 1),
                            skip_group_check=True)
                    ob = wpool.tile([OUT_DIM, CHUNK], F32, tag=f"ob{c}",
                                    name=f"ob{c}", bufs=2)
                    nc.scalar.activation(ob[:], yT, ACTF.Identity,
                                         bias=cy_sb[:, 0:1])
                    col0 = p * BP + c * CHUNK
                    nc.sync.dma_start(out_d.ap()[:, col0:col0 + CHUNK],
                                      ob[:])

    nc.compile()
    return nc


_CACHED = None
RUN_KWARGS = {}
LAST_RESULT = None


def _get_nc():
    global _CACHED
    if _CACHED is None:
        _CACHED = _build()
    return _CACHED


def kernel(x, Wx, bx, W, U, b, tau, Wf, bf):
    x = np.asarray(x, np.float32)
    Wx = np.asarray(Wx, np.float64)
    bx = np.asarray(bx, np.float64)
    W = np.asarray(W, np.float64)
    U = np.asarray(U, np.float64)
    b = np.asarray(b, np.float64)
    tau = np.asarray(tau, np.float64)
    Wf = np.asarray(Wf, np.float64)
    bf = np.asarray(bf, np.float64)

    # ---- ETDRK3 (Cox-Matthews) coefficients, dt = 1, L = -1/tau ----
    z = -1.0 / tau
    z2 = 0.5 * z
    E2 = np.exp(z2)
    E = np.exp(z)

    def phi1(v): return np.expm1(v) / v
    def phi2(v): return (np.expm1(v) - v) / v**2
    def phi3(v): return (np.expm1(v) - v - v * v / 2) / v**3

    gam2 = 0.5 * phi1(z2) / tau
    gam1 = phi1(z) / tau
    g1 = (4 * phi3(z) - 3 * phi2(z) + phi1(z)) / tau
    g2 = (4 * phi2(z) - 8 * phi3(z)) / tau
    g3 = (4 * phi3(z) - phi2(z)) / tau

    M1 = (W + U) @ Wx
    c1 = (W + U) @ bx
    M2 = (W * (E2 - 1.0)[None, :]) @ Wx
    c2 = W @ ((E2 - 1.0) * bx)
    M3 = (W * (E - E2)[None, :]) @ Wx
    c3 = W @ ((E - E2) * bx)
    Mh = (Wf * E[None, :]) @ Wx
    cy = Wf @ (E * bx) + bf
    Wg2 = W * gam2[None, :]
    Wg1 = W * (2.0 * gam1)[None, :]       # the "2 t2" factor folded in
    Wf1 = Wf * g1[None, :]
    Wfd = Wf * g2[None, :]                # head group carries g2
    r = 0.5 * (gam1 + gam2) / gam1        # so v2 = t2 - r t1

    bias1 = b + c1
    bias2 = bias1 + c2
    bias3 = bias2 + c3

    weights = {
        "wg1": _pack_lhsT(Wg1.T.astype(np.float16)),
        "wg2": _pack_lhsT(Wg2.T.astype(np.float16)),
        "m1": _pack_lhsT(M1.T.astype(np.float16)),
        "m2": _pack_lhsT(M2.T.astype(np.float16)),
        "m3": _pack_lhsT(M3.T.astype(np.float16)),
        "wf": _pack_lhsT(Wfd.T.astype(np.float16)),
        "wf1": _pack_lhsT(Wf1.T.astype(np.float16)),
        "mh": _pack_lhsT(Mh.T.astype(np.float16)),
        "bias1": _pack_pp(bias1.astype(np.float32)),
        "bias2": _pack_pp(bias2.astype(np.float32)),
        "bias3": _pack_pp(bias3.astype(np.float32)),
        "rbc": _bcast(r),
        "g3bc": _bcast(g3 / g2),
        "cy": np.ascontiguousarray(cy.astype(np.float32).reshape(128, 1)),
    }

    nc = _get_nc()
    in_maps = []
    for c in range(N_CORES):
        m = dict(weights)
        m["xT"] = np.ascontiguousarray(
            x[c * BL:(c + 1) * BL].T.astype(np.float16))
        in_maps.append(m)
    res = bass_utils.run_bass_kernel_spmd(nc, in_maps,
                                          core_ids=list(range(N_CORES)),
                                          **RUN_KWARGS)
    global LAST_RESULT
    LAST_RESULT = res
    return np.ascontiguousarray(
        np.concatenate([res.results[c]["out"].T for c in range(N_CORES)],
                       axis=0))
